# revision 26
# baseline (speedup 1.0000x reference)
"""Trainium2 Bass kernel for nn_AttnBlock (GroupNorm + single-head attention over
32x32 image tokens + residual), batch 32, C=512, distributed data-parallel over
8 NeuronCores (4 images per core, no collectives).

Per-image pipeline on each core (all GEMMs fp16 inputs / fp32 PSUM accumulate):
  x[c,n] --groupnorm--> hn[c,n] (fp16)
  q[o,n] = wq @ hn ; k[o,m] = wk @ hn            (lhsT = host-transposed weights)
  vT[m,c] = hn^T @ wv^T                           (produced pre-transposed)
  sT[m,n] = k^T q ; eT = exp(sT/sqrt(C))          (softmax max-subtraction skipped:
                                                   softmax is shift invariant and
                                                   |s| <= ~6 so exp is fp32-safe)
  rowsum[n] = ones^T @ eT  (PE, every output partition = rowsum -> free bcast)
  out[c,n] = vT^T @ eT     (unnormalized)
  y = x + (wp @ out) * (1/rowsum) + bp
"""

import os
import sys

import numpy as np

for _p in ("/opt/trn_rl_repo", "/root/.axon_site/_ro/trn_rl_repo"):
    if os.path.isdir(_p) and _p not in sys.path:
        sys.path.append(_p)

from contextlib import ExitStack

import concourse.bass as bass  # noqa: E402
import concourse.tile as tile  # noqa: E402
from concourse import bacc, mybir  # noqa: E402
from concourse.bass_utils import run_bass_kernel_spmd  # noqa: E402

P = 128
B, C, H, W = 32, 512, 32, 32
N = H * W                  # 1024 tokens per image
CO = C // P                # 4 channel slabs of 128
FD = 512                   # matmul free-dim chunk (one PSUM bank of fp32)
NCH = N // FD              # 2 free-dim chunks
MO = N // P                # 8 token slabs of 128
GROUPS = 16
EPS = 1e-6
NCORES = 8
IPC = B // NCORES          # images per core
F32 = mybir.dt.float32
F16 = mybir.dt.float16
AF = mybir.ActivationFunctionType
OP = mybir.AluOpType
SCALE = float(C) ** -0.5


def _emit(tc: "tile.TileContext", ctx: ExitStack, aps: dict):
    nc = tc.nc

    const = ctx.enter_context(tc.tile_pool(name="const", bufs=1))
    xs = ctx.enter_context(tc.tile_pool(name="xs", bufs=2))
    hns = ctx.enter_context(tc.tile_pool(name="hns", bufs=2))
    qs = ctx.enter_context(tc.tile_pool(name="qs", bufs=1))
    ks = ctx.enter_context(tc.tile_pool(name="ks", bufs=1))
    vs = ctx.enter_context(tc.tile_pool(name="vs", bufs=1))
    es = ctx.enter_context(tc.tile_pool(name="es", bufs=1))
    ous = ctx.enter_context(tc.tile_pool(name="ous", bufs=1))
    ris = ctx.enter_context(tc.tile_pool(name="ris", bufs=2))
    ys = ctx.enter_context(tc.tile_pool(name="ys", bufs=3))
    stat = ctx.enter_context(tc.tile_pool(name="stat", bufs=2))
    mmp = ctx.enter_context(tc.tile_pool(name="mmp", bufs=6, space="PSUM"))
    smp = ctx.enter_context(tc.tile_pool(name="smp", bufs=1, space="PSUM"))
    wmp = ctx.enter_context(tc.tile_pool(name="wmp", bufs=1, space="PSUM"))

    # ---- constants: one packed DMA on the GpSimd queue so the Sync queue
    # is free for the critical-path x slabs ----
    ones_sb = const.tile([P, P], F16, tag="ones")
    nc.vector.memset(ones_sb[:], 1.0)
    cpack = const.tile([P, 5 * CO + P + C], F32, tag="cpack")
    nc.gpsimd.dma_start(cpack[:], aps["cpack"])
    small = {}
    for i, name in enumerate(("bq", "bk", "bp", "gamma", "beta")):
        small[name] = cpack[:, i * CO : (i + 1) * CO]
    proj_sb = cpack[:, 5 * CO : 5 * CO + P]
    bvb_sb = cpack[:, 5 * CO + P :]

    # Dummy matmuls while groupnorm owns the critical path: PE is idle anyway
    # and sustained activity lifts the HAM clock gate to 8/8 before real work.
    wps = wmp.tile([P, P], F32, tag="warm")

    def warmup(n):
        for i in range(n):
            nc.tensor.matmul(
                wps[:], lhsT=ones_sb[:], rhs=ones_sb[:], start=(i == 0), stop=(i == n - 1)
            )

    w_sb = {}

    def load_weights():
        # Emitted after prep(0) so x(0) slabs go first on the DMA queue;
        # wqT leads since the first projection matmuls consume it.
        for name in ("wqT", "wkT", "wvT", "wpT"):
            t = const.tile([P, CO, C], F16, tag=name)
            nc.sync.dma_start(t[:], aps[name].rearrange("(co ci) o -> ci co o", ci=P))
            w_sb[name] = t

    # Per-image state carried between the pipeline stages below.
    st = [dict() for _ in range(IPC)]

    def prep(img):
        """x DMA + groupnorm -> hn (DVE/ACT work; one tiny PE matmul).

        Emitted one image ahead of its consumer so the DVE chain overlaps the
        previous image's attention matmuls.
        """
        x_ap = aps["x"][img].rearrange("(co ci) n -> ci co n", ci=P)
        x_sb = xs.tile([P, CO, N], F32, tag="x")
        stats = stat.tile([P, 2 * CO], F32, tag="stats")
        for co in range(CO):
            nc.sync.dma_start(x_sb[:, co], x_ap[:, co])
            # sum(x) on DVE, sum(x^2) on ACT (Square + free-dim accumulator)
            # run concurrently; the group projector folds the 1/(32*1024).
            nc.vector.reduce_sum(
                stats[:, co : co + 1], x_sb[:, co], axis=mybir.AxisListType.X
            )
            scr = stat.tile([P, N], F16, tag="sqscr")
            nc.scalar.activation(
                scr[:],
                x_sb[:, co],
                AF.Square,
                accum_out=stats[:, CO + co : CO + co + 1],
            )
        gs_ps = smp.tile([P, 2 * CO], F32, tag="gs")
        nc.tensor.matmul(gs_ps[:], lhsT=proj_sb[:], rhs=stats[:], start=True, stop=True)
        gs = stat.tile([P, 2 * CO], F32, tag="gss")
        nc.scalar.activation(gs[:], gs_ps[:], AF.Copy)
        m2 = stat.tile([P, CO], F32, tag="m2")
        nc.vector.tensor_mul(m2[:], gs[:, 0:CO], gs[:, 0:CO])
        varg = stat.tile([P, CO], F32, tag="varg")
        nc.vector.tensor_sub(varg[:], gs[:, CO : 2 * CO], m2[:])
        # rstd = 1/sqrt(var+eps) entirely on DVE (quake rsqrt + 3 Newton
        # steps, ~1e-10 rel err) so ACT only ever uses the exp table.
        ve = stat.tile([P, CO], F32, tag="ve")
        nc.vector.tensor_scalar(
            out=ve[:], in0=varg[:], scalar1=EPS, scalar2=None, op0=OP.add
        )
        y0i = stat.tile([P, CO], mybir.dt.int32, tag="y0i")
        nc.vector.tensor_scalar(
            out=y0i[:],
            in0=ve[:].bitcast(mybir.dt.int32),
            scalar1=1,
            scalar2=None,
            op0=OP.arith_shift_right,
        )
        nc.vector.tensor_scalar(
            out=y0i[:],
            in0=y0i[:],
            scalar1=-1,
            scalar2=0x5F3759DF,
            op0=OP.mult,
            op1=OP.add,
        )
        rstd = y0i[:].bitcast(F32)
        for _ in range(2):
            yy = stat.tile([P, CO], F32, tag="yy")
            nc.vector.tensor_mul(yy[:], rstd, rstd)
            nc.vector.tensor_mul(yy[:], yy[:], ve[:])
            nc.vector.tensor_scalar(
                out=yy[:], in0=yy[:], scalar1=-0.5, scalar2=1.5, op0=OP.mult, op1=OP.add
            )
            nxt = stat.tile([P, CO], F32, tag="rstd")
            nc.vector.tensor_mul(nxt[:], rstd, yy[:])
            rstd = nxt[:]
        a_sc = stat.tile([P, CO], F32, tag="a_sc")
        nc.vector.tensor_mul(a_sc[:], small["gamma"][:], rstd[:])
        bt = stat.tile([P, CO], F32, tag="bt")
        nc.vector.tensor_mul(bt[:], gs[:, 0:CO], a_sc[:])
        b_sc = stat.tile([P, CO], F32, tag="b_sc")
        nc.vector.tensor_sub(b_sc[:], small["beta"][:], bt[:])

        hn = hns.tile([P, CO, N], F16, tag="hn")
        for co in range(CO):
            nc.vector.tensor_scalar(
                out=hn[:, co],
                in0=x_sb[:, co],
                scalar1=a_sc[:, co : co + 1],
                scalar2=b_sc[:, co : co + 1],
                op0=OP.mult,
                op1=OP.add,
            )
        st[img]["x"] = x_sb
        st[img]["hn"] = hn

    def head(img):
        """q/k projections, vT, scores + exp (the first ~60% of PE work)."""
        hn = st[img]["hn"]
        q_sb = qs.tile([P, CO, N], F16, tag="q")
        k_sb = ks.tile([P, CO, N], F16, tag="k")
        for wname, dst, bname in (("wqT", q_sb, "bq"), ("wkT", k_sb, "bk")):
            wt = w_sb[wname]
            for ot in range(CO):
                for ch in range(NCH):
                    ps = mmp.tile([P, FD], F32, tag="mm")
                    for ci in range(CO):
                        nc.tensor.matmul(
                            ps[:],
                            lhsT=wt[:, ci, ot * P : (ot + 1) * P],
                            rhs=hn[:, ci, ch * FD : (ch + 1) * FD],
                            start=(ci == 0),
                            stop=(ci == CO - 1),
                        )
                    nc.scalar.activation(
                        dst[:, ot, ch * FD : (ch + 1) * FD],
                        ps[:],
                        AF.Identity,
                        bias=small[bname][:, ot : ot + 1],
                    )

        vT = vs.tile([P, MO, C], F16, tag="vT")
        for mt in range(MO):
            ps = mmp.tile([P, FD], F32, tag="mm")
            for ci in range(CO):
                nc.tensor.matmul(
                    ps[:],
                    lhsT=hn[:, ci, mt * P : (mt + 1) * P],
                    rhs=w_sb["wvT"][:, ci, :],
                    start=(ci == 0),
                    stop=(ci == CO - 1),
                )
            nc.vector.tensor_add(vT[:, mt], ps[:], bvb_sb[:])

        eT = es.tile([P, MO, N], F16, tag="eT")
        for mt in range(MO):
            for ch in range(NCH):
                ps = mmp.tile([P, FD], F32, tag="mm")
                for oo in range(CO):
                    nc.tensor.matmul(
                        ps[:],
                        lhsT=k_sb[:, oo, mt * P : (mt + 1) * P],
                        rhs=q_sb[:, oo, ch * FD : (ch + 1) * FD],
                        start=(oo == 0),
                        stop=(oo == CO - 1),
                    )
                nc.scalar.activation(
                    eT[:, mt, ch * FD : (ch + 1) * FD], ps[:], AF.Exp, scale=SCALE
                )
        st[img]["vT"] = vT
        st[img]["eT"] = eT

    def tail(img):
        """out GEMM, rowsum, proj + residual.  out before rowsum so the PE
        never waits on the last exp evictions; y MMs overlap the reciprocal."""
        x_sb, vT, eT = st[img]["x"], st[img]["vT"], st[img]["eT"]
        y_ap = aps["y"][img].rearrange("(co ci) n -> ci co n", ci=P)

        out_sb = ous.tile([P, CO, N], F16, tag="out")
        for ct in range(CO):
            for ch in range(NCH):
                ps = mmp.tile([P, FD], F32, tag="mm")
                for mt in range(MO):
                    nc.tensor.matmul(
                        ps[:],
                        lhsT=vT[:, mt, ct * P : (ct + 1) * P],
                        rhs=eT[:, mt, ch * FD : (ch + 1) * FD],
                        start=(mt == 0),
                        stop=(mt == MO - 1),
                    )
                nc.scalar.activation(out_sb[:, ct, ch * FD : (ch + 1) * FD], ps[:], AF.Copy)

        rinv = ris.tile([P, N], F32, tag="rinv")
        for ch in range(NCH):
            ps = mmp.tile([P, FD], F32, tag="mm")
            for mt in range(MO):
                nc.tensor.matmul(
                    ps[:],
                    lhsT=ones_sb[:],
                    rhs=eT[:, mt, ch * FD : (ch + 1) * FD],
                    start=(mt == 0),
                    stop=(mt == MO - 1),
                )
            rscr = ys.tile([P, FD], F32, tag="rscr")
            nc.vector.reciprocal_approx_accurate(
                rinv[:, ch * FD : (ch + 1) * FD], ps[:], rscr[:]
            )

        for ot in range(CO):
            for ch in range(NCH):
                ps = mmp.tile([P, FD], F32, tag="mm")
                for ci in range(CO):
                    nc.tensor.matmul(
                        ps[:],
                        lhsT=w_sb["wpT"][:, ci, ot * P : (ot + 1) * P],
                        rhs=out_sb[:, ci, ch * FD : (ch + 1) * FD],
                        start=(ci == 0),
                        stop=(ci == CO - 1),
                    )
                t1 = ys.tile([P, FD], F32, tag="yt")
                nc.vector.tensor_mul(t1[:], ps[:], rinv[:, ch * FD : (ch + 1) * FD])
                t2 = ys.tile([P, FD], F32, tag="yo")
                nc.vector.scalar_tensor_tensor(
                    out=t2[:],
                    in0=t1[:],
                    scalar=small["bp"][:, ot : ot + 1],
                    in1=x_sb[:, ot, ch * FD : (ch + 1) * FD],
                    op0=OP.add,
                    op1=OP.add,
                )
                nc.sync.dma_start(y_ap[:, ot, ch * FD : (ch + 1) * FD], t2[:])

    warmup(150)
    prep(0)
    warmup(90)
    wsb = stat.tile([P, P], F32, tag="warm_sb")
    nc.scalar.activation(wsb[:], wps[:], AF.Copy)
    nc.gpsimd.dma_start(aps["wsink"], wsb[:])
    load_weights()
    for img in range(IPC):
        head(img)
        if img + 1 < IPC:
            prep(img + 1)
        tail(img)


def _build_program():
    nc = bacc.Bacc("TRN2", target_bir_lowering=False, debug=False)
    aps = {}
    aps["x"] = nc.dram_tensor("x", [IPC, C, N], F32, kind="ExternalInput").ap()
    for name in ("wqT", "wkT", "wvT", "wpT"):
        aps[name] = nc.dram_tensor(name, [C, C], F16, kind="ExternalInput").ap()
    aps["cpack"] = nc.dram_tensor(
        "cpack", [P, 5 * CO + P + C], F32, kind="ExternalInput"
    ).ap()
    aps["y"] = nc.dram_tensor("y", [IPC, C, N], F32, kind="ExternalOutput").ap()
    aps["wsink"] = nc.dram_tensor("wsink", [P, P], F32, kind="ExternalOutput").ap()

    with tile.TileContext(nc) as tc:
        with ExitStack() as ctx:
            _emit(tc, ctx, aps)
    nc.compile()
    return nc


_PROGRAM = None


def _get_program():
    global _PROGRAM
    if _PROGRAM is None:
        _PROGRAM = _build_program()
    return _PROGRAM


def _col_layout(v):
    # (C,) vector -> [128, CO] tile layout with c = co*128 + ci at [ci, co]
    return np.ascontiguousarray(v.reshape(CO, P).T.astype(np.float32))


def _make_in_maps(inputs):
    x = np.asarray(inputs["x"], dtype=np.float32).reshape(B, C, N)
    cpack = np.concatenate(
        [
            _col_layout(np.asarray(inputs["bq"])),
            _col_layout(np.asarray(inputs["bk"])),
            _col_layout(np.asarray(inputs["bp"])),
            _col_layout(np.asarray(inputs["gn_gamma"])),
            _col_layout(np.asarray(inputs["gn_beta"])),
            _make_proj(),
            np.tile(np.asarray(inputs["bv"], dtype=np.float32)[None, :], (P, 1)),
        ],
        axis=1,
    )
    shared = {
        "wqT": np.ascontiguousarray(np.asarray(inputs["wq"]).T.astype(np.float16)),
        "wkT": np.ascontiguousarray(np.asarray(inputs["wk"]).T.astype(np.float16)),
        "wvT": np.ascontiguousarray(np.asarray(inputs["wv"]).T.astype(np.float16)),
        "wpT": np.ascontiguousarray(np.asarray(inputs["wp"]).T.astype(np.float16)),
        "cpack": np.ascontiguousarray(cpack),
    }
    in_maps = []
    for core in range(NCORES):
        m = dict(shared)
        m["x"] = np.ascontiguousarray(x[core * IPC : (core + 1) * IPC])
        in_maps.append(m)
    return in_maps


def _make_proj():
    # [128,128] group-averaging projector applied to raw (sum, sumsq) rows:
    # P[i,j] = (i//32 == j//32) / (32*N)  (channel c = co*128 + ci; each co
    # slab holds 4 groups of 32 channels; stats are sums over N pixels)
    gsz = P // (GROUPS // CO)  # 32
    idx = np.arange(P) // gsz
    return np.ascontiguousarray(
        (idx[:, None] == idx[None, :]).astype(np.float32) / (gsz * N)
    )


def _run(inputs, trace=False):
    nc = _get_program()
    in_maps = _make_in_maps(inputs)
    res = run_bass_kernel_spmd(nc, in_maps, core_ids=list(range(NCORES)), trace=trace)
    y = np.concatenate([r["y"] for r in res.results], axis=0)  # (B, C, N)
    return y.reshape(B, C, H, W).astype(np.float32), res.exec_time_ns


def kernel(**inputs):
    return _run(inputs, trace=False)[0]


# revision 28
# speedup vs baseline: 1.0030x; 1.0030x over previous
"""Trainium2 Bass kernel for nn_AttnBlock (GroupNorm + single-head attention over
32x32 image tokens + residual), batch 32, C=512, distributed data-parallel over
8 NeuronCores (4 images per core, no collectives).

Per-image pipeline on each core (all GEMMs fp16 inputs / fp32 PSUM accumulate):
  x[c,n] --groupnorm--> hn[c,n] (fp16)
  q[o,n] = wq @ hn ; k[o,m] = wk @ hn            (lhsT = host-transposed weights)
  vT[m,c] = hn^T @ wv^T                           (produced pre-transposed)
  sT[m,n] = k^T q ; eT = exp(sT/sqrt(C))          (softmax max-subtraction skipped:
                                                   softmax is shift invariant and
                                                   |s| <= ~6 so exp is fp32-safe)
  rowsum[n] = ones^T @ eT  (PE, every output partition = rowsum -> free bcast)
  out[c,n] = vT^T @ eT     (unnormalized)
  y = x + (wp @ out) * (1/rowsum) + bp
"""

import os
import sys

import numpy as np

for _p in ("/opt/trn_rl_repo", "/root/.axon_site/_ro/trn_rl_repo"):
    if os.path.isdir(_p) and _p not in sys.path:
        sys.path.append(_p)

from contextlib import ExitStack

import concourse.bass as bass  # noqa: E402
import concourse.tile as tile  # noqa: E402
from concourse import bacc, mybir  # noqa: E402
from concourse.bass_utils import run_bass_kernel_spmd  # noqa: E402

P = 128
B, C, H, W = 32, 512, 32, 32
N = H * W                  # 1024 tokens per image
CO = C // P                # 4 channel slabs of 128
FD = 512                   # matmul free-dim chunk (one PSUM bank of fp32)
NCH = N // FD              # 2 free-dim chunks
MO = N // P                # 8 token slabs of 128
GROUPS = 16
EPS = 1e-6
NCORES = 8
IPC = B // NCORES          # images per core
F32 = mybir.dt.float32
F16 = mybir.dt.float16
AF = mybir.ActivationFunctionType
OP = mybir.AluOpType
SCALE = float(C) ** -0.5


def _emit(tc: "tile.TileContext", ctx: ExitStack, aps: dict):
    nc = tc.nc

    const = ctx.enter_context(tc.tile_pool(name="const", bufs=1))
    xs = ctx.enter_context(tc.tile_pool(name="xs", bufs=2))
    hns = ctx.enter_context(tc.tile_pool(name="hns", bufs=2))
    qs = ctx.enter_context(tc.tile_pool(name="qs", bufs=1))
    ks = ctx.enter_context(tc.tile_pool(name="ks", bufs=1))
    vs = ctx.enter_context(tc.tile_pool(name="vs", bufs=1))
    es = ctx.enter_context(tc.tile_pool(name="es", bufs=1))
    ous = ctx.enter_context(tc.tile_pool(name="ous", bufs=1))
    ris = ctx.enter_context(tc.tile_pool(name="ris", bufs=2))
    ys = ctx.enter_context(tc.tile_pool(name="ys", bufs=3))
    stat = ctx.enter_context(tc.tile_pool(name="stat", bufs=2))
    mmp = ctx.enter_context(tc.tile_pool(name="mmp", bufs=6, space="PSUM"))
    smp = ctx.enter_context(tc.tile_pool(name="smp", bufs=1, space="PSUM"))
    wmp = ctx.enter_context(tc.tile_pool(name="wmp", bufs=1, space="PSUM"))

    # ---- constants: one packed DMA on the GpSimd queue so the Sync queue
    # is free for the critical-path x slabs ----
    ones_sb = const.tile([P, P], F16, tag="ones")
    nc.vector.memset(ones_sb[:], 1.0)
    cpack = const.tile([P, 5 * CO + P + C], F32, tag="cpack")
    nc.gpsimd.dma_start(cpack[:], aps["cpack"])
    small = {}
    for i, name in enumerate(("bq", "bk", "bp", "gamma", "beta")):
        small[name] = cpack[:, i * CO : (i + 1) * CO]
    proj_sb = cpack[:, 5 * CO : 5 * CO + P]
    bvb_sb = cpack[:, 5 * CO + P :]

    # Dummy matmuls while groupnorm owns the critical path: PE is idle anyway
    # and sustained activity lifts the HAM clock gate to 8/8 before real work.
    wps = wmp.tile([P, P], F32, tag="warm")

    def warmup(n):
        for i in range(n):
            nc.tensor.matmul(
                wps[:], lhsT=ones_sb[:], rhs=ones_sb[:], start=(i == 0), stop=(i == n - 1)
            )

    w_sb = {}

    def load_weights():
        # Emitted after prep(0) so x(0) slabs go first on the DMA queue;
        # wqT leads since the first projection matmuls consume it.
        for name in ("wqT", "wkT", "wvT", "wpT"):
            t = const.tile([P, CO, C], F16, tag=name)
            nc.sync.dma_start(t[:], aps[name].rearrange("(co ci) o -> ci co o", ci=P))
            w_sb[name] = t

    # Per-image state carried between the pipeline stages below.
    st = [dict() for _ in range(IPC)]

    def prep(img):
        """x DMA + groupnorm -> hn (DVE/ACT work; one tiny PE matmul).

        Emitted one image ahead of its consumer so the DVE/ACT chain overlaps
        the previous image's attention matmuls.  rstd = 1/sqrt(var+eps) runs
        on DVE (quake-style rsqrt + Newton) so the ACT engine only ever needs
        one activation table (exp/copy/identity/square) -> one table load.
        """
        x_ap = aps["x"][img].rearrange("(co ci) n -> ci co n", ci=P)
        x_sb = xs.tile([P, CO, N], F32, tag="x")
        stats = stat.tile([P, 2 * CO], F32, tag="stats")
        for co in range(CO):
            nc.sync.dma_start(x_sb[:, co], x_ap[:, co])
            # sum(x) on DVE, sum(x^2) on ACT (Square + free-dim accumulator)
            # run concurrently; the group projector folds the 1/(32*1024).
            nc.vector.reduce_sum(
                stats[:, co : co + 1], x_sb[:, co], axis=mybir.AxisListType.X
            )
            scr = stat.tile([P, N], F16, tag="sqscr")
            nc.scalar.activation(
                scr[:],
                x_sb[:, co],
                AF.Square,
                accum_out=stats[:, CO + co : CO + co + 1],
            )
        gs_ps = smp.tile([P, 2 * CO], F32, tag="gs")
        nc.tensor.matmul(gs_ps[:], lhsT=proj_sb[:], rhs=stats[:], start=True, stop=True)
        gs = stat.tile([P, 2 * CO], F32, tag="gss")
        nc.scalar.activation(gs[:], gs_ps[:], AF.Copy)
        m2 = stat.tile([P, CO], F32, tag="m2")
        nc.vector.tensor_mul(m2[:], gs[:, 0:CO], gs[:, 0:CO])
        varg = stat.tile([P, CO], F32, tag="varg")
        nc.vector.tensor_sub(varg[:], gs[:, CO : 2 * CO], m2[:])
        # rstd = 1/sqrt(var+eps) entirely on DVE (quake rsqrt + 3 Newton
        # steps, ~1e-10 rel err) so ACT only ever uses the exp table.
        ve = stat.tile([P, CO], F32, tag="ve")
        nc.vector.tensor_scalar(
            out=ve[:], in0=varg[:], scalar1=EPS, scalar2=None, op0=OP.add
        )
        y0i = stat.tile([P, CO], mybir.dt.int32, tag="y0i")
        nc.vector.tensor_scalar(
            out=y0i[:],
            in0=ve[:].bitcast(mybir.dt.int32),
            scalar1=1,
            scalar2=None,
            op0=OP.arith_shift_right,
        )
        nc.vector.tensor_scalar(
            out=y0i[:],
            in0=y0i[:],
            scalar1=-1,
            scalar2=0x5F3759DF,
            op0=OP.mult,
            op1=OP.add,
        )
        rstd = y0i[:].bitcast(F32)
        for _ in range(2):
            yy = stat.tile([P, CO], F32, tag="yy")
            nc.vector.tensor_mul(yy[:], rstd, rstd)
            nc.vector.tensor_mul(yy[:], yy[:], ve[:])
            nc.vector.tensor_scalar(
                out=yy[:], in0=yy[:], scalar1=-0.5, scalar2=1.5, op0=OP.mult, op1=OP.add
            )
            nxt = stat.tile([P, CO], F32, tag="rstd")
            nc.vector.tensor_mul(nxt[:], rstd, yy[:])
            rstd = nxt[:]
        a_sc = stat.tile([P, CO], F32, tag="a_sc")
        nc.vector.tensor_mul(a_sc[:], small["gamma"][:], rstd[:])
        bt = stat.tile([P, CO], F32, tag="bt")
        nc.vector.tensor_mul(bt[:], gs[:, 0:CO], a_sc[:])
        b_sc = stat.tile([P, CO], F32, tag="b_sc")
        nc.vector.tensor_sub(b_sc[:], small["beta"][:], bt[:])

        hn = hns.tile([P, CO, N], F16, tag="hn")
        for co in range(CO):
            nc.vector.tensor_scalar(
                out=hn[:, co],
                in0=x_sb[:, co],
                scalar1=a_sc[:, co : co + 1],
                scalar2=b_sc[:, co : co + 1],
                op0=OP.mult,
                op1=OP.add,
            )
        st[img]["x"] = x_sb
        st[img]["hn"] = hn

    def head(img):
        """q/k projections, vT, scores + exp (the first ~60% of PE work)."""
        hn = st[img]["hn"]
        q_sb = qs.tile([P, CO, N], F16, tag="q")
        k_sb = ks.tile([P, CO, N], F16, tag="k")
        for wname, dst, bname in (("wqT", q_sb, "bq"), ("wkT", k_sb, "bk")):
            wt = w_sb[wname]
            for ot in range(CO):
                for ch in range(NCH):
                    ps = mmp.tile([P, FD], F32, tag="mm")
                    for ci in range(CO):
                        nc.tensor.matmul(
                            ps[:],
                            lhsT=wt[:, ci, ot * P : (ot + 1) * P],
                            rhs=hn[:, ci, ch * FD : (ch + 1) * FD],
                            start=(ci == 0),
                            stop=(ci == CO - 1),
                        )
                    nc.scalar.activation(
                        dst[:, ot, ch * FD : (ch + 1) * FD],
                        ps[:],
                        AF.Identity,
                        bias=small[bname][:, ot : ot + 1],
                    )

        vT = vs.tile([P, MO, C], F16, tag="vT")
        for mt in range(MO):
            ps = mmp.tile([P, FD], F32, tag="mm")
            for ci in range(CO):
                nc.tensor.matmul(
                    ps[:],
                    lhsT=hn[:, ci, mt * P : (mt + 1) * P],
                    rhs=w_sb["wvT"][:, ci, :],
                    start=(ci == 0),
                    stop=(ci == CO - 1),
                )
            nc.vector.tensor_add(vT[:, mt], ps[:], bvb_sb[:])

        eT = es.tile([P, MO, N], F16, tag="eT")
        for mt in range(MO):
            for ch in range(NCH):
                ps = mmp.tile([P, FD], F32, tag="mm")
                for oo in range(CO):
                    nc.tensor.matmul(
                        ps[:],
                        lhsT=k_sb[:, oo, mt * P : (mt + 1) * P],
                        rhs=q_sb[:, oo, ch * FD : (ch + 1) * FD],
                        start=(oo == 0),
                        stop=(oo == CO - 1),
                    )
                nc.scalar.activation(
                    eT[:, mt, ch * FD : (ch + 1) * FD], ps[:], AF.Exp, scale=SCALE
                )
        st[img]["vT"] = vT
        st[img]["eT"] = eT

    def tail(img):
        """out GEMM, rowsum, proj + residual.  out before rowsum so the PE
        never waits on the last exp evictions; y MMs overlap the reciprocal."""
        x_sb, vT, eT = st[img]["x"], st[img]["vT"], st[img]["eT"]
        y_ap = aps["y"][img].rearrange("(co ci) n -> ci co n", ci=P)

        out_sb = ous.tile([P, CO, N], F16, tag="out")
        for ct in range(CO):
            for ch in range(NCH):
                ps = mmp.tile([P, FD], F32, tag="mm")
                for mt in range(MO):
                    nc.tensor.matmul(
                        ps[:],
                        lhsT=vT[:, mt, ct * P : (ct + 1) * P],
                        rhs=eT[:, mt, ch * FD : (ch + 1) * FD],
                        start=(mt == 0),
                        stop=(mt == MO - 1),
                    )
                nc.scalar.activation(out_sb[:, ct, ch * FD : (ch + 1) * FD], ps[:], AF.Copy)

        rinv = ris.tile([P, N], F32, tag="rinv")
        for ch in range(NCH):
            ps = mmp.tile([P, FD], F32, tag="mm")
            for mt in range(MO):
                nc.tensor.matmul(
                    ps[:],
                    lhsT=ones_sb[:],
                    rhs=eT[:, mt, ch * FD : (ch + 1) * FD],
                    start=(mt == 0),
                    stop=(mt == MO - 1),
                )
            rscr = ys.tile([P, FD], F32, tag="rscr")
            nc.vector.reciprocal_approx_accurate(
                rinv[:, ch * FD : (ch + 1) * FD], ps[:], rscr[:]
            )

        for ot in range(CO):
            for ch in range(NCH):
                ps = mmp.tile([P, FD], F32, tag="mm")
                for ci in range(CO):
                    nc.tensor.matmul(
                        ps[:],
                        lhsT=w_sb["wpT"][:, ci, ot * P : (ot + 1) * P],
                        rhs=out_sb[:, ci, ch * FD : (ch + 1) * FD],
                        start=(ci == 0),
                        stop=(ci == CO - 1),
                    )
                t1 = ys.tile([P, FD], F32, tag="yt")
                nc.vector.tensor_mul(t1[:], ps[:], rinv[:, ch * FD : (ch + 1) * FD])
                t2 = ys.tile([P, FD], F32, tag="yo")
                nc.vector.scalar_tensor_tensor(
                    out=t2[:],
                    in0=t1[:],
                    scalar=small["bp"][:, ot : ot + 1],
                    in1=x_sb[:, ot, ch * FD : (ch + 1) * FD],
                    op0=OP.add,
                    op1=OP.add,
                )
                nc.sync.dma_start(y_ap[:, ot, ch * FD : (ch + 1) * FD], t2[:])

    warmup(130)
    prep(0)
    warmup(90)
    wsb = stat.tile([P, P], F32, tag="warm_sb")
    nc.scalar.activation(wsb[:], wps[:], AF.Copy)
    nc.gpsimd.dma_start(aps["wsink"], wsb[:])
    load_weights()
    for img in range(IPC):
        head(img)
        if img + 1 < IPC:
            prep(img + 1)
        tail(img)


def _build_program():
    nc = bacc.Bacc("TRN2", target_bir_lowering=False, debug=False)
    aps = {}
    aps["x"] = nc.dram_tensor("x", [IPC, C, N], F32, kind="ExternalInput").ap()
    for name in ("wqT", "wkT", "wvT", "wpT"):
        aps[name] = nc.dram_tensor(name, [C, C], F16, kind="ExternalInput").ap()
    aps["cpack"] = nc.dram_tensor(
        "cpack", [P, 5 * CO + P + C], F32, kind="ExternalInput"
    ).ap()
    aps["y"] = nc.dram_tensor("y", [IPC, C, N], F32, kind="ExternalOutput").ap()
    aps["wsink"] = nc.dram_tensor("wsink", [P, P], F32, kind="ExternalOutput").ap()

    with tile.TileContext(nc) as tc:
        with ExitStack() as ctx:
            _emit(tc, ctx, aps)
    nc.compile()
    return nc


_PROGRAM = None


def _get_program():
    global _PROGRAM
    if _PROGRAM is None:
        _PROGRAM = _build_program()
    return _PROGRAM


def _col_layout(v):
    # (C,) vector -> [128, CO] tile layout with c = co*128 + ci at [ci, co]
    return np.ascontiguousarray(v.reshape(CO, P).T.astype(np.float32))


def _make_in_maps(inputs):
    x = np.asarray(inputs["x"], dtype=np.float32).reshape(B, C, N)
    cpack = np.concatenate(
        [
            _col_layout(np.asarray(inputs["bq"])),
            _col_layout(np.asarray(inputs["bk"])),
            _col_layout(np.asarray(inputs["bp"])),
            _col_layout(np.asarray(inputs["gn_gamma"])),
            _col_layout(np.asarray(inputs["gn_beta"])),
            _make_proj(),
            np.tile(np.asarray(inputs["bv"], dtype=np.float32)[None, :], (P, 1)),
        ],
        axis=1,
    )
    shared = {
        "wqT": np.ascontiguousarray(np.asarray(inputs["wq"]).T.astype(np.float16)),
        "wkT": np.ascontiguousarray(np.asarray(inputs["wk"]).T.astype(np.float16)),
        "wvT": np.ascontiguousarray(np.asarray(inputs["wv"]).T.astype(np.float16)),
        "wpT": np.ascontiguousarray(np.asarray(inputs["wp"]).T.astype(np.float16)),
        "cpack": np.ascontiguousarray(cpack),
    }
    in_maps = []
    for core in range(NCORES):
        m = dict(shared)
        m["x"] = np.ascontiguousarray(x[core * IPC : (core + 1) * IPC])
        in_maps.append(m)
    return in_maps


def _make_proj():
    # [128,128] group-averaging projector applied to raw (sum, sumsq) rows:
    # P[i,j] = (i//32 == j//32) / (32*N)  (channel c = co*128 + ci; each co
    # slab holds 4 groups of 32 channels; stats are sums over N pixels)
    gsz = P // (GROUPS // CO)  # 32
    idx = np.arange(P) // gsz
    return np.ascontiguousarray(
        (idx[:, None] == idx[None, :]).astype(np.float32) / (gsz * N)
    )


def _run(inputs, trace=False):
    nc = _get_program()
    in_maps = _make_in_maps(inputs)
    res = run_bass_kernel_spmd(nc, in_maps, core_ids=list(range(NCORES)), trace=trace)
    y = np.concatenate([r["y"] for r in res.results], axis=0)  # (B, C, N)
    return y.reshape(B, C, H, W).astype(np.float32), res.exec_time_ns


def kernel(**inputs):
    return _run(inputs, trace=False)[0]


# revision 30
# speedup vs baseline: 1.0037x; 1.0006x over previous
"""Trainium2 Bass kernel for nn_AttnBlock (GroupNorm + single-head attention over
32x32 image tokens + residual), batch 32, C=512, distributed data-parallel over
8 NeuronCores (4 images per core, no collectives).

Per-image pipeline on each core (all GEMMs fp16 inputs / fp32 PSUM accumulate):
  x[c,n] --groupnorm--> hn[c,n] (fp16)
  q[o,n] = wq @ hn ; k[o,m] = wk @ hn            (lhsT = host-transposed weights)
  vT[m,c] = hn^T @ wv^T                           (produced pre-transposed)
  sT[m,n] = k^T q ; eT = exp(sT/sqrt(C))          (softmax max-subtraction skipped:
                                                   softmax is shift invariant and
                                                   |s| <= ~6 so exp is fp32-safe)
  rowsum[n] = ones^T @ eT  (PE, every output partition = rowsum -> free bcast)
  out[c,n] = vT^T @ eT     (unnormalized)
  y = x + (wp @ out) * (1/rowsum) + bp
"""

import os
import sys

import numpy as np

for _p in ("/opt/trn_rl_repo", "/root/.axon_site/_ro/trn_rl_repo"):
    if os.path.isdir(_p) and _p not in sys.path:
        sys.path.append(_p)

from contextlib import ExitStack

import concourse.tile as tile  # noqa: E402
from concourse import bacc, mybir  # noqa: E402
from concourse.bass_utils import run_bass_kernel_spmd  # noqa: E402

P = 128
B, C, H, W = 32, 512, 32, 32
N = H * W                  # 1024 tokens per image
CO = C // P                # 4 channel slabs of 128
FD = 512                   # matmul free-dim chunk (one PSUM bank of fp32)
NCH = N // FD              # 2 free-dim chunks
MO = N // P                # 8 token slabs of 128
GROUPS = 16
EPS = 1e-6
NCORES = 8
IPC = B // NCORES          # images per core
F32 = mybir.dt.float32
F16 = mybir.dt.float16
AF = mybir.ActivationFunctionType
OP = mybir.AluOpType
SCALE = float(C) ** -0.5


def _emit(tc: "tile.TileContext", ctx: ExitStack, aps: dict):
    nc = tc.nc

    const = ctx.enter_context(tc.tile_pool(name="const", bufs=1))
    xs = ctx.enter_context(tc.tile_pool(name="xs", bufs=2))
    hns = ctx.enter_context(tc.tile_pool(name="hns", bufs=2))
    qs = ctx.enter_context(tc.tile_pool(name="qs", bufs=1))
    ks = ctx.enter_context(tc.tile_pool(name="ks", bufs=1))
    vs = ctx.enter_context(tc.tile_pool(name="vs", bufs=1))
    es = ctx.enter_context(tc.tile_pool(name="es", bufs=1))
    ous = ctx.enter_context(tc.tile_pool(name="ous", bufs=1))
    ris = ctx.enter_context(tc.tile_pool(name="ris", bufs=2))
    ys = ctx.enter_context(tc.tile_pool(name="ys", bufs=3))
    stat = ctx.enter_context(tc.tile_pool(name="stat", bufs=2))
    mmp = ctx.enter_context(tc.tile_pool(name="mmp", bufs=6, space="PSUM"))
    smp = ctx.enter_context(tc.tile_pool(name="smp", bufs=1, space="PSUM"))
    wmp = ctx.enter_context(tc.tile_pool(name="wmp", bufs=1, space="PSUM"))

    # ---- constants: one packed DMA on the GpSimd queue so the Sync queue
    # is free for the critical-path x slabs ----
    ones_sb = const.tile([P, P], F16, tag="ones")
    nc.vector.memset(ones_sb[:], 1.0)
    cpack = const.tile([P, 5 * CO + P + C], F32, tag="cpack")
    nc.gpsimd.dma_start(cpack[:], aps["cpack"])
    small = {}
    for i, name in enumerate(("bq", "bk", "bp", "gamma", "beta")):
        small[name] = cpack[:, i * CO : (i + 1) * CO]
    proj_sb = cpack[:, 5 * CO : 5 * CO + P]
    bvb_sb = cpack[:, 5 * CO + P :]

    # Dummy matmuls while groupnorm owns the critical path: PE is idle anyway
    # and sustained activity lifts the HAM clock gate to 8/8 before real work.
    wps = wmp.tile([P, P], F32, tag="warm")

    def warmup(n):
        for i in range(n):
            nc.tensor.matmul(
                wps[:], lhsT=ones_sb[:], rhs=ones_sb[:], start=(i == 0), stop=(i == n - 1)
            )

    w_sb = {}

    def load_weights():
        # Emitted after prep(0) so x(0) slabs go first on the DMA queue;
        # wqT leads since the first projection matmuls consume it.
        for name in ("wqT", "wkT", "wvT", "wpT"):
            t = const.tile([P, CO, C], F16, tag=name)
            nc.sync.dma_start(t[:], aps[name].rearrange("(co ci) o -> ci co o", ci=P))
            w_sb[name] = t

    # Per-image state carried between the pipeline stages below.
    st = [dict() for _ in range(IPC)]

    def prep(img):
        """x DMA + groupnorm -> hn (DVE/ACT work; one tiny PE matmul).

        Emitted one image ahead of its consumer so the DVE/ACT chain overlaps
        the previous image's attention matmuls.  rstd = 1/sqrt(var+eps) runs
        on DVE (quake-style rsqrt + Newton) so the ACT engine only ever needs
        one activation table (exp/copy/identity/square) -> one table load.
        """
        x_ap = aps["x"][img].rearrange("(co ci) n -> ci co n", ci=P)
        x_sb = xs.tile([P, CO, N], F32, tag="x")
        stats = stat.tile([P, 2 * CO], F32, tag="stats")
        for co in range(CO):
            nc.sync.dma_start(x_sb[:, co], x_ap[:, co])
            # sum(x) on DVE, sum(x^2) on ACT (Square + free-dim accumulator)
            # run concurrently; the group projector folds the 1/(32*1024).
            nc.vector.reduce_sum(
                stats[:, co : co + 1], x_sb[:, co], axis=mybir.AxisListType.X
            )
            scr = stat.tile([P, N], F16, tag="sqscr")
            nc.scalar.activation(
                scr[:],
                x_sb[:, co],
                AF.Square,
                accum_out=stats[:, CO + co : CO + co + 1],
            )
        gs_ps = smp.tile([P, 2 * CO], F32, tag="gs")
        nc.tensor.matmul(gs_ps[:], lhsT=proj_sb[:], rhs=stats[:], start=True, stop=True)
        gs = stat.tile([P, 2 * CO], F32, tag="gss")
        nc.scalar.activation(gs[:], gs_ps[:], AF.Copy)
        m2 = stat.tile([P, CO], F32, tag="m2")
        nc.vector.tensor_mul(m2[:], gs[:, 0:CO], gs[:, 0:CO])
        varg = stat.tile([P, CO], F32, tag="varg")
        nc.vector.tensor_sub(varg[:], gs[:, CO : 2 * CO], m2[:])
        # rstd = 1/sqrt(var+eps) entirely on DVE (quake rsqrt + 2 Newton
        # steps, ~5e-6 rel err) so ACT only ever uses the exp table.
        ve = stat.tile([P, CO], F32, tag="ve")
        nc.vector.tensor_scalar(
            out=ve[:], in0=varg[:], scalar1=EPS, scalar2=None, op0=OP.add
        )
        y0i = stat.tile([P, CO], mybir.dt.int32, tag="y0i")
        nc.vector.tensor_scalar(
            out=y0i[:],
            in0=ve[:].bitcast(mybir.dt.int32),
            scalar1=1,
            scalar2=None,
            op0=OP.arith_shift_right,
        )
        nc.vector.tensor_scalar(
            out=y0i[:],
            in0=y0i[:],
            scalar1=-1,
            scalar2=0x5F3759DF,
            op0=OP.mult,
            op1=OP.add,
        )
        rstd = y0i[:].bitcast(F32)
        for _ in range(2):
            yy = stat.tile([P, CO], F32, tag="yy")
            nc.vector.tensor_mul(yy[:], rstd, rstd)
            nc.vector.tensor_mul(yy[:], yy[:], ve[:])
            nc.vector.tensor_scalar(
                out=yy[:], in0=yy[:], scalar1=-0.5, scalar2=1.5, op0=OP.mult, op1=OP.add
            )
            nxt = stat.tile([P, CO], F32, tag="rstd")
            nc.vector.tensor_mul(nxt[:], rstd, yy[:])
            rstd = nxt[:]
        a_sc = stat.tile([P, CO], F32, tag="a_sc")
        nc.vector.tensor_mul(a_sc[:], small["gamma"][:], rstd[:])
        bt = stat.tile([P, CO], F32, tag="bt")
        nc.vector.tensor_mul(bt[:], gs[:, 0:CO], a_sc[:])
        b_sc = stat.tile([P, CO], F32, tag="b_sc")
        nc.vector.tensor_sub(b_sc[:], small["beta"][:], bt[:])

        hn = hns.tile([P, CO, N], F16, tag="hn")
        for co in range(CO):
            nc.vector.tensor_scalar(
                out=hn[:, co],
                in0=x_sb[:, co],
                scalar1=a_sc[:, co : co + 1],
                scalar2=b_sc[:, co : co + 1],
                op0=OP.mult,
                op1=OP.add,
            )
        st[img]["x"] = x_sb
        st[img]["hn"] = hn

    def head(img):
        """q/k projections, vT, scores + exp (the first ~60% of PE work)."""
        hn = st[img]["hn"]
        q_sb = qs.tile([P, CO, N], F16, tag="q")
        k_sb = ks.tile([P, CO, N], F16, tag="k")
        for wname, dst, bname in (("wqT", q_sb, "bq"), ("wkT", k_sb, "bk")):
            wt = w_sb[wname]
            for ot in range(CO):
                for ch in range(NCH):
                    ps = mmp.tile([P, FD], F32, tag="mm")
                    for ci in range(CO):
                        nc.tensor.matmul(
                            ps[:],
                            lhsT=wt[:, ci, ot * P : (ot + 1) * P],
                            rhs=hn[:, ci, ch * FD : (ch + 1) * FD],
                            start=(ci == 0),
                            stop=(ci == CO - 1),
                        )
                    nc.scalar.activation(
                        dst[:, ot, ch * FD : (ch + 1) * FD],
                        ps[:],
                        AF.Identity,
                        bias=small[bname][:, ot : ot + 1],
                    )

        vT = vs.tile([P, MO, C], F16, tag="vT")
        for mt in range(MO):
            ps = mmp.tile([P, FD], F32, tag="mm")
            for ci in range(CO):
                nc.tensor.matmul(
                    ps[:],
                    lhsT=hn[:, ci, mt * P : (mt + 1) * P],
                    rhs=w_sb["wvT"][:, ci, :],
                    start=(ci == 0),
                    stop=(ci == CO - 1),
                )
            nc.vector.tensor_add(vT[:, mt], ps[:], bvb_sb[:])

        eT = es.tile([P, MO, N], F16, tag="eT")
        for mt in range(MO):
            for ch in range(NCH):
                ps = mmp.tile([P, FD], F32, tag="mm")
                for oo in range(CO):
                    nc.tensor.matmul(
                        ps[:],
                        lhsT=k_sb[:, oo, mt * P : (mt + 1) * P],
                        rhs=q_sb[:, oo, ch * FD : (ch + 1) * FD],
                        start=(oo == 0),
                        stop=(oo == CO - 1),
                    )
                nc.scalar.activation(
                    eT[:, mt, ch * FD : (ch + 1) * FD], ps[:], AF.Exp, scale=SCALE
                )
        st[img]["vT"] = vT
        st[img]["eT"] = eT

    def tail(img):
        """out GEMM, rowsum, proj + residual.  out before rowsum so the PE
        never waits on the last exp evictions; y MMs overlap the reciprocal."""
        x_sb, vT, eT = st[img]["x"], st[img]["vT"], st[img]["eT"]
        y_ap = aps["y"][img].rearrange("(co ci) n -> ci co n", ci=P)

        out_sb = ous.tile([P, CO, N], F16, tag="out")
        for ct in range(CO):
            for ch in range(NCH):
                ps = mmp.tile([P, FD], F32, tag="mm")
                for mt in range(MO):
                    nc.tensor.matmul(
                        ps[:],
                        lhsT=vT[:, mt, ct * P : (ct + 1) * P],
                        rhs=eT[:, mt, ch * FD : (ch + 1) * FD],
                        start=(mt == 0),
                        stop=(mt == MO - 1),
                    )
                nc.scalar.activation(out_sb[:, ct, ch * FD : (ch + 1) * FD], ps[:], AF.Copy)

        rinv = ris.tile([P, N], F32, tag="rinv")
        for ch in range(NCH):
            ps = mmp.tile([P, FD], F32, tag="mm")
            for mt in range(MO):
                nc.tensor.matmul(
                    ps[:],
                    lhsT=ones_sb[:],
                    rhs=eT[:, mt, ch * FD : (ch + 1) * FD],
                    start=(mt == 0),
                    stop=(mt == MO - 1),
                )
            rscr = ys.tile([P, FD], F32, tag="rscr")
            nc.vector.reciprocal_approx_accurate(
                rinv[:, ch * FD : (ch + 1) * FD], ps[:], rscr[:]
            )

        for ot in range(CO):
            for ch in range(NCH):
                ps = mmp.tile([P, FD], F32, tag="mm")
                for ci in range(CO):
                    nc.tensor.matmul(
                        ps[:],
                        lhsT=w_sb["wpT"][:, ci, ot * P : (ot + 1) * P],
                        rhs=out_sb[:, ci, ch * FD : (ch + 1) * FD],
                        start=(ci == 0),
                        stop=(ci == CO - 1),
                    )
                t1 = ys.tile([P, FD], F32, tag="yt")
                nc.vector.tensor_mul(t1[:], ps[:], rinv[:, ch * FD : (ch + 1) * FD])
                t2 = ys.tile([P, FD], F32, tag="yo")
                nc.vector.scalar_tensor_tensor(
                    out=t2[:],
                    in0=t1[:],
                    scalar=small["bp"][:, ot : ot + 1],
                    in1=x_sb[:, ot, ch * FD : (ch + 1) * FD],
                    op0=OP.add,
                    op1=OP.add,
                )
                nc.sync.dma_start(y_ap[:, ot, ch * FD : (ch + 1) * FD], t2[:])

    warmup(130)
    prep(0)
    warmup(90)
    wsb = stat.tile([P, P], F32, tag="warm_sb")
    nc.scalar.activation(wsb[:], wps[:], AF.Copy)
    nc.gpsimd.dma_start(aps["wsink"], wsb[:])
    load_weights()
    for img in range(IPC):
        head(img)
        if img + 1 < IPC:
            prep(img + 1)
        tail(img)


def _build_program():
    nc = bacc.Bacc("TRN2", target_bir_lowering=False, debug=False)
    aps = {}
    aps["x"] = nc.dram_tensor("x", [IPC, C, N], F32, kind="ExternalInput").ap()
    for name in ("wqT", "wkT", "wvT", "wpT"):
        aps[name] = nc.dram_tensor(name, [C, C], F16, kind="ExternalInput").ap()
    aps["cpack"] = nc.dram_tensor(
        "cpack", [P, 5 * CO + P + C], F32, kind="ExternalInput"
    ).ap()
    aps["y"] = nc.dram_tensor("y", [IPC, C, N], F32, kind="ExternalOutput").ap()
    aps["wsink"] = nc.dram_tensor("wsink", [P, P], F32, kind="ExternalOutput").ap()

    with tile.TileContext(nc) as tc:
        with ExitStack() as ctx:
            _emit(tc, ctx, aps)
    nc.compile()
    return nc


_PROGRAM = None


def _get_program():
    global _PROGRAM
    if _PROGRAM is None:
        _PROGRAM = _build_program()
    return _PROGRAM


def _col_layout(v):
    # (C,) vector -> [128, CO] tile layout with c = co*128 + ci at [ci, co]
    return np.ascontiguousarray(v.reshape(CO, P).T.astype(np.float32))


def _make_in_maps(inputs):
    x = np.asarray(inputs["x"], dtype=np.float32).reshape(B, C, N)
    cpack = np.concatenate(
        [
            _col_layout(np.asarray(inputs["bq"])),
            _col_layout(np.asarray(inputs["bk"])),
            _col_layout(np.asarray(inputs["bp"])),
            _col_layout(np.asarray(inputs["gn_gamma"])),
            _col_layout(np.asarray(inputs["gn_beta"])),
            _make_proj(),
            np.tile(np.asarray(inputs["bv"], dtype=np.float32)[None, :], (P, 1)),
        ],
        axis=1,
    )
    shared = {
        "wqT": np.ascontiguousarray(np.asarray(inputs["wq"]).T.astype(np.float16)),
        "wkT": np.ascontiguousarray(np.asarray(inputs["wk"]).T.astype(np.float16)),
        "wvT": np.ascontiguousarray(np.asarray(inputs["wv"]).T.astype(np.float16)),
        "wpT": np.ascontiguousarray(np.asarray(inputs["wp"]).T.astype(np.float16)),
        "cpack": np.ascontiguousarray(cpack),
    }
    in_maps = []
    for core in range(NCORES):
        m = dict(shared)
        m["x"] = np.ascontiguousarray(x[core * IPC : (core + 1) * IPC])
        in_maps.append(m)
    return in_maps


def _make_proj():
    # [128,128] group-averaging projector applied to raw (sum, sumsq) rows:
    # P[i,j] = (i//32 == j//32) / (32*N)  (channel c = co*128 + ci; each co
    # slab holds 4 groups of 32 channels; stats are sums over N pixels)
    gsz = P // (GROUPS // CO)  # 32
    idx = np.arange(P) // gsz
    return np.ascontiguousarray(
        (idx[:, None] == idx[None, :]).astype(np.float32) / (gsz * N)
    )


def _run(inputs, trace=False):
    nc = _get_program()
    in_maps = _make_in_maps(inputs)
    res = run_bass_kernel_spmd(nc, in_maps, core_ids=list(range(NCORES)), trace=trace)
    y = np.concatenate([r["y"] for r in res.results], axis=0)  # (B, C, N)
    return y.reshape(B, C, H, W).astype(np.float32), res.exec_time_ns


def kernel(**inputs):
    return _run(inputs, trace=False)[0]


# revision 35
# speedup vs baseline: 1.0260x; 1.0222x over previous
"""Trainium2 Bass kernel for nn_AttnBlock (GroupNorm + single-head attention over
32x32 image tokens + residual), batch 32, C=512, distributed data-parallel over
8 NeuronCores (4 images per core, no collectives).

Per-image pipeline on each core (all GEMMs fp16 inputs / fp32 PSUM accumulate):
  x[c,n] --groupnorm--> hn[c,n] (fp16)
  q[o,n] = wq @ hn ; k[o,m] = wk @ hn            (lhsT = host-transposed weights)
  vT[m,c] = hn^T @ wv^T                           (produced pre-transposed)
  sT[m,n] = k^T q ; eT = exp(sT/sqrt(C))          (softmax max-subtraction skipped:
                                                   softmax is shift invariant and
                                                   |s| <= ~6 so exp is fp32-safe)
  rowsum[n] = ones^T @ eT  (PE, every output partition = rowsum -> free bcast)
  out[c,n] = vT^T @ eT     (unnormalized)
  y = x + (wp @ out) * (1/rowsum) + bp
"""

import os
import sys

import numpy as np

for _p in ("/opt/trn_rl_repo", "/root/.axon_site/_ro/trn_rl_repo"):
    if os.path.isdir(_p) and _p not in sys.path:
        sys.path.append(_p)

from contextlib import ExitStack

import concourse.tile as tile  # noqa: E402
from concourse import bacc, mybir  # noqa: E402
from concourse.bass_utils import run_bass_kernel_spmd  # noqa: E402

P = 128
B, C, H, W = 32, 512, 32, 32
N = H * W                  # 1024 tokens per image
CO = C // P                # 4 channel slabs of 128
FD = 512                   # matmul free-dim chunk (one PSUM bank of fp32)
NCH = N // FD              # 2 free-dim chunks
MO = N // P                # 8 token slabs of 128
GROUPS = 16
EPS = 1e-6
NCORES = 8
IPC = B // NCORES          # images per core
F32 = mybir.dt.float32
F16 = mybir.dt.float16
AF = mybir.ActivationFunctionType
OP = mybir.AluOpType
SCALE = float(C) ** -0.5


def _emit(tc: "tile.TileContext", ctx: ExitStack, aps: dict):
    nc = tc.nc

    const = ctx.enter_context(tc.tile_pool(name="const", bufs=1))
    xs = ctx.enter_context(tc.tile_pool(name="xs", bufs=2))
    hns = ctx.enter_context(tc.tile_pool(name="hns", bufs=2))
    qs = ctx.enter_context(tc.tile_pool(name="qs", bufs=1))
    ks = ctx.enter_context(tc.tile_pool(name="ks", bufs=1))
    vs = ctx.enter_context(tc.tile_pool(name="vs", bufs=1))
    es = ctx.enter_context(tc.tile_pool(name="es", bufs=1))
    ous = ctx.enter_context(tc.tile_pool(name="ous", bufs=1))
    ris = ctx.enter_context(tc.tile_pool(name="ris", bufs=2))
    accp = ctx.enter_context(tc.tile_pool(name="accp", bufs=2))
    ys = ctx.enter_context(tc.tile_pool(name="ys", bufs=3))
    stat = ctx.enter_context(tc.tile_pool(name="stat", bufs=2))
    mmp = ctx.enter_context(tc.tile_pool(name="mmp", bufs=6, space="PSUM"))
    smp = ctx.enter_context(tc.tile_pool(name="smp", bufs=1, space="PSUM"))
    wmp = ctx.enter_context(tc.tile_pool(name="wmp", bufs=1, space="PSUM"))

    # ---- constants: one packed DMA on the GpSimd queue so the Sync queue
    # is free for the critical-path x slabs ----
    ones_sb = const.tile([P, P], F16, tag="ones")
    nc.vector.memset(ones_sb[:], 1.0)
    ones32_sb = const.tile([P, P], F32, tag="ones32")
    nc.vector.memset(ones32_sb[:], 1.0)
    cpack = const.tile([P, 5 * CO + P + C], F32, tag="cpack")
    nc.gpsimd.dma_start(cpack[:], aps["cpack"])
    small = {}
    for i, name in enumerate(("bq", "bk", "bp", "gamma", "beta")):
        small[name] = cpack[:, i * CO : (i + 1) * CO]
    proj_sb = cpack[:, 5 * CO : 5 * CO + P]
    bvb_sb = cpack[:, 5 * CO + P :]

    # Dummy matmuls while groupnorm owns the critical path: PE is idle anyway
    # and sustained activity lifts the HAM clock gate to 8/8 before real work.
    wps = wmp.tile([P, P], F32, tag="warm")

    def warmup(n):
        for i in range(n):
            nc.tensor.matmul(
                wps[:], lhsT=ones_sb[:], rhs=ones_sb[:], start=(i == 0), stop=(i == n - 1)
            )

    w_sb = {}

    def load_weights():
        # Emitted after prep(0) so x(0) slabs go first on the DMA queue;
        # wqT leads since the first projection matmuls consume it.
        for name in ("wqT", "wkT", "wvT", "wpT"):
            t = const.tile([P, CO, C], F16, tag=name)
            nc.sync.dma_start(t[:], aps[name].rearrange("(co ci) o -> ci co o", ci=P))
            w_sb[name] = t

    # Per-image state carried between the pipeline stages below.
    st = [dict() for _ in range(IPC)]

    def prep(img):
        """x DMA + groupnorm -> hn (DVE/ACT work; one tiny PE matmul).

        Emitted one image ahead of its consumer so the DVE/ACT chain overlaps
        the previous image's attention matmuls.  rstd = 1/sqrt(var+eps) runs
        on DVE (quake-style rsqrt + Newton) so the ACT engine only ever needs
        one activation table (exp/copy/identity/square) -> one table load.
        """
        x_ap = aps["x"][img].rearrange("(co ci) n -> ci co n", ci=P)
        x_sb = xs.tile([P, CO, N], F32, tag="x")
        stats = stat.tile([P, 2 * CO], F32, tag="stats")
        for co in range(CO):
            nc.sync.dma_start(x_sb[:, co], x_ap[:, co])
            # sum(x) on DVE, sum(x^2) on ACT (Square + free-dim accumulator)
            # run concurrently; the group projector folds the 1/(32*1024).
            nc.vector.reduce_sum(
                stats[:, co : co + 1], x_sb[:, co], axis=mybir.AxisListType.X
            )
            scr = stat.tile([P, N], F16, tag="sqscr")
            nc.scalar.activation(
                scr[:],
                x_sb[:, co],
                AF.Square,
                accum_out=stats[:, CO + co : CO + co + 1],
            )
        gs_ps = smp.tile([P, 2 * CO], F32, tag="gs")
        nc.tensor.matmul(gs_ps[:], lhsT=proj_sb[:], rhs=stats[:], start=True, stop=True)
        gs = stat.tile([P, 2 * CO], F32, tag="gss")
        nc.scalar.activation(gs[:], gs_ps[:], AF.Copy)
        m2 = stat.tile([P, CO], F32, tag="m2")
        nc.vector.tensor_mul(m2[:], gs[:, 0:CO], gs[:, 0:CO])
        varg = stat.tile([P, CO], F32, tag="varg")
        nc.vector.tensor_sub(varg[:], gs[:, CO : 2 * CO], m2[:])
        # rstd = 1/sqrt(var+eps) entirely on DVE (quake rsqrt + 2 Newton
        # steps, ~5e-6 rel err) so ACT only ever uses the exp table.
        ve = stat.tile([P, CO], F32, tag="ve")
        nc.vector.tensor_scalar(
            out=ve[:], in0=varg[:], scalar1=EPS, scalar2=None, op0=OP.add
        )
        y0i = stat.tile([P, CO], mybir.dt.int32, tag="y0i")
        nc.vector.tensor_scalar(
            out=y0i[:],
            in0=ve[:].bitcast(mybir.dt.int32),
            scalar1=1,
            scalar2=None,
            op0=OP.arith_shift_right,
        )
        nc.vector.tensor_scalar(
            out=y0i[:],
            in0=y0i[:],
            scalar1=-1,
            scalar2=0x5F3759DF,
            op0=OP.mult,
            op1=OP.add,
        )
        rstd = y0i[:].bitcast(F32)
        for _ in range(2):
            yy = stat.tile([P, CO], F32, tag="yy")
            nc.vector.tensor_mul(yy[:], rstd, rstd)
            nc.vector.tensor_mul(yy[:], yy[:], ve[:])
            nc.vector.tensor_scalar(
                out=yy[:], in0=yy[:], scalar1=-0.5, scalar2=1.5, op0=OP.mult, op1=OP.add
            )
            nxt = stat.tile([P, CO], F32, tag="rstd")
            nc.vector.tensor_mul(nxt[:], rstd, yy[:])
            rstd = nxt[:]
        a_sc = stat.tile([P, CO], F32, tag="a_sc")
        nc.vector.tensor_mul(a_sc[:], small["gamma"][:], rstd[:])
        bt = stat.tile([P, CO], F32, tag="bt")
        nc.vector.tensor_mul(bt[:], gs[:, 0:CO], a_sc[:])
        b_sc = stat.tile([P, CO], F32, tag="b_sc")
        nc.vector.tensor_sub(b_sc[:], small["beta"][:], bt[:])

        hn = hns.tile([P, CO, N], F16, tag="hn")
        for co in range(CO):
            nc.vector.tensor_scalar(
                out=hn[:, co],
                in0=x_sb[:, co],
                scalar1=a_sc[:, co : co + 1],
                scalar2=b_sc[:, co : co + 1],
                op0=OP.mult,
                op1=OP.add,
            )
        st[img]["x"] = x_sb
        st[img]["hn"] = hn

    def head(img):
        """q/k projections, vT, scores + exp (the first ~60% of PE work)."""
        hn = st[img]["hn"]
        q_sb = qs.tile([P, CO, N], F16, tag="q")
        k_sb = ks.tile([P, CO, N], F16, tag="k")
        for wname, dst, bname in (("wqT", q_sb, "bq"), ("wkT", k_sb, "bk")):
            wt = w_sb[wname]
            for ot in range(CO):
                for ch in range(NCH):
                    ps = mmp.tile([P, FD], F32, tag="mm")
                    for ci in range(CO):
                        nc.tensor.matmul(
                            ps[:],
                            lhsT=wt[:, ci, ot * P : (ot + 1) * P],
                            rhs=hn[:, ci, ch * FD : (ch + 1) * FD],
                            start=(ci == 0),
                            stop=(ci == CO - 1),
                        )
                    nc.scalar.activation(
                        dst[:, ot, ch * FD : (ch + 1) * FD],
                        ps[:],
                        AF.Identity,
                        bias=small[bname][:, ot : ot + 1],
                    )

        vT = vs.tile([P, MO, C], F16, tag="vT")
        for mt in range(MO):
            ps = mmp.tile([P, FD], F32, tag="mm")
            for ci in range(CO):
                nc.tensor.matmul(
                    ps[:],
                    lhsT=hn[:, ci, mt * P : (mt + 1) * P],
                    rhs=w_sb["wvT"][:, ci, :],
                    start=(ci == 0),
                    stop=(ci == CO - 1),
                )
            nc.vector.tensor_add(vT[:, mt], ps[:], bvb_sb[:])

        eT = es.tile([P, MO, N], F16, tag="eT")
        for mt in range(MO):
            for ch in range(NCH):
                ps = mmp.tile([P, FD], F32, tag="mm")
                for oo in range(CO):
                    nc.tensor.matmul(
                        ps[:],
                        lhsT=k_sb[:, oo, mt * P : (mt + 1) * P],
                        rhs=q_sb[:, oo, ch * FD : (ch + 1) * FD],
                        start=(oo == 0),
                        stop=(oo == CO - 1),
                    )
                nc.scalar.activation(
                    eT[:, mt, ch * FD : (ch + 1) * FD], ps[:], AF.Exp, scale=SCALE
                )
        # Fold the 8 m-slabs of eT on the (otherwise idle) GpSimd engine so the
        # rowsum needs only one ones-matmul per chunk instead of eight.
        acc = accp.tile([P, N], F32, tag="acc")
        nc.gpsimd.tensor_add(acc[:], eT[:, 0], eT[:, 1])
        for mt in range(2, MO):
            nc.gpsimd.tensor_add(acc[:], acc[:], eT[:, mt])
        st[img]["vT"] = vT
        st[img]["eT"] = eT
        st[img]["acc"] = acc

    def tail(img):
        """out GEMM, rowsum, proj + residual.  out before rowsum so the PE
        never waits on the last exp evictions; y MMs overlap the reciprocal."""
        x_sb, vT, eT = st[img]["x"], st[img]["vT"], st[img]["eT"]
        y_ap = aps["y"][img].rearrange("(co ci) n -> ci co n", ci=P)

        out_sb = ous.tile([P, CO, N], F16, tag="out")
        for ct in range(CO):
            for ch in range(NCH):
                ps = mmp.tile([P, FD], F32, tag="mm")
                for mt in range(MO):
                    nc.tensor.matmul(
                        ps[:],
                        lhsT=vT[:, mt, ct * P : (ct + 1) * P],
                        rhs=eT[:, mt, ch * FD : (ch + 1) * FD],
                        start=(mt == 0),
                        stop=(mt == MO - 1),
                    )
                nc.scalar.activation(out_sb[:, ct, ch * FD : (ch + 1) * FD], ps[:], AF.Copy)

        acc = st[img]["acc"]
        rinv = ris.tile([P, N], F32, tag="rinv")
        for ch in range(NCH):
            ps = mmp.tile([P, FD], F32, tag="mm")
            nc.tensor.matmul(
                ps[:],
                lhsT=ones32_sb[:],
                rhs=acc[:, ch * FD : (ch + 1) * FD],
                start=True,
                stop=True,
            )
            rscr = ys.tile([P, FD], F32, tag="rscr")
            nc.vector.reciprocal_approx_accurate(
                rinv[:, ch * FD : (ch + 1) * FD], ps[:], rscr[:]
            )

        for ot in range(CO):
            for ch in range(NCH):
                ps = mmp.tile([P, FD], F32, tag="mm")
                for ci in range(CO):
                    nc.tensor.matmul(
                        ps[:],
                        lhsT=w_sb["wpT"][:, ci, ot * P : (ot + 1) * P],
                        rhs=out_sb[:, ci, ch * FD : (ch + 1) * FD],
                        start=(ci == 0),
                        stop=(ci == CO - 1),
                    )
                t1 = ys.tile([P, FD], F32, tag="yt")
                nc.vector.tensor_mul(t1[:], ps[:], rinv[:, ch * FD : (ch + 1) * FD])
                t2 = ys.tile([P, FD], F32, tag="yo")
                nc.vector.scalar_tensor_tensor(
                    out=t2[:],
                    in0=t1[:],
                    scalar=small["bp"][:, ot : ot + 1],
                    in1=x_sb[:, ot, ch * FD : (ch + 1) * FD],
                    op0=OP.add,
                    op1=OP.add,
                )
                nc.sync.dma_start(y_ap[:, ot, ch * FD : (ch + 1) * FD], t2[:])

    warmup(130)
    prep(0)
    warmup(90)
    wsb = stat.tile([P, P], F32, tag="warm_sb")
    nc.scalar.activation(wsb[:], wps[:], AF.Copy)
    nc.gpsimd.dma_start(aps["wsink"], wsb[:])
    load_weights()
    for img in range(IPC):
        head(img)
        if img + 1 < IPC:
            prep(img + 1)
        tail(img)


def _build_program():
    nc = bacc.Bacc("TRN2", target_bir_lowering=False, debug=False)
    aps = {}
    aps["x"] = nc.dram_tensor("x", [IPC, C, N], F32, kind="ExternalInput").ap()
    for name in ("wqT", "wkT", "wvT", "wpT"):
        aps[name] = nc.dram_tensor(name, [C, C], F16, kind="ExternalInput").ap()
    aps["cpack"] = nc.dram_tensor(
        "cpack", [P, 5 * CO + P + C], F32, kind="ExternalInput"
    ).ap()
    aps["y"] = nc.dram_tensor("y", [IPC, C, N], F32, kind="ExternalOutput").ap()
    aps["wsink"] = nc.dram_tensor("wsink", [P, P], F32, kind="ExternalOutput").ap()

    with tile.TileContext(nc) as tc:
        with ExitStack() as ctx:
            _emit(tc, ctx, aps)
    nc.compile()
    return nc


_PROGRAM = None


def _get_program():
    global _PROGRAM
    if _PROGRAM is None:
        _PROGRAM = _build_program()
    return _PROGRAM


def _col_layout(v):
    # (C,) vector -> [128, CO] tile layout with c = co*128 + ci at [ci, co]
    return np.ascontiguousarray(v.reshape(CO, P).T.astype(np.float32))


def _make_in_maps(inputs):
    x = np.asarray(inputs["x"], dtype=np.float32).reshape(B, C, N)
    cpack = np.concatenate(
        [
            _col_layout(np.asarray(inputs["bq"])),
            _col_layout(np.asarray(inputs["bk"])),
            _col_layout(np.asarray(inputs["bp"])),
            _col_layout(np.asarray(inputs["gn_gamma"])),
            _col_layout(np.asarray(inputs["gn_beta"])),
            _make_proj(),
            np.tile(np.asarray(inputs["bv"], dtype=np.float32)[None, :], (P, 1)),
        ],
        axis=1,
    )
    shared = {
        "wqT": np.ascontiguousarray(np.asarray(inputs["wq"]).T.astype(np.float16)),
        "wkT": np.ascontiguousarray(np.asarray(inputs["wk"]).T.astype(np.float16)),
        "wvT": np.ascontiguousarray(np.asarray(inputs["wv"]).T.astype(np.float16)),
        "wpT": np.ascontiguousarray(np.asarray(inputs["wp"]).T.astype(np.float16)),
        "cpack": np.ascontiguousarray(cpack),
    }
    in_maps = []
    for core in range(NCORES):
        m = dict(shared)
        m["x"] = np.ascontiguousarray(x[core * IPC : (core + 1) * IPC])
        in_maps.append(m)
    return in_maps


def _make_proj():
    # [128,128] group-averaging projector applied to raw (sum, sumsq) rows:
    # P[i,j] = (i//32 == j//32) / (32*N)  (channel c = co*128 + ci; each co
    # slab holds 4 groups of 32 channels; stats are sums over N pixels)
    gsz = P // (GROUPS // CO)  # 32
    idx = np.arange(P) // gsz
    return np.ascontiguousarray(
        (idx[:, None] == idx[None, :]).astype(np.float32) / (gsz * N)
    )


def _run(inputs, trace=False):
    nc = _get_program()
    in_maps = _make_in_maps(inputs)
    res = run_bass_kernel_spmd(nc, in_maps, core_ids=list(range(NCORES)), trace=trace)
    y = np.concatenate([r["y"] for r in res.results], axis=0)  # (B, C, N)
    return y.reshape(B, C, H, W).astype(np.float32), res.exec_time_ns


def kernel(**inputs):
    return _run(inputs, trace=False)[0]


# revision 40
# speedup vs baseline: 1.0424x; 1.0160x over previous
"""Trainium2 Bass kernel for nn_AttnBlock (GroupNorm + single-head attention over
32x32 image tokens + residual), batch 32, C=512, distributed data-parallel over
8 NeuronCores (4 images per core, no collectives).

Per-image pipeline on each core (all GEMMs fp16 inputs / fp32 PSUM accumulate):
  x[c,n] --groupnorm--> hn[c,n] (fp16)
  q[o,n] = wq @ hn ; k[o,m] = wk @ hn            (lhsT = host-transposed weights)
  vT[m,c] = hn^T @ wv^T                           (produced pre-transposed)
  sT[m,n] = k^T q ; eT = exp(sT/sqrt(C))          (softmax max-subtraction skipped:
                                                   softmax is shift invariant and
                                                   |s| <= ~6 so exp is fp32-safe)
  rowsum[n] = ones^T @ eT  (PE, every output partition = rowsum -> free bcast)
  out[c,n] = vT^T @ eT     (unnormalized)
  y = x + (wp @ out) * (1/rowsum) + bp
"""

import os
import sys

import numpy as np

for _p in ("/opt/trn_rl_repo", "/root/.axon_site/_ro/trn_rl_repo"):
    if os.path.isdir(_p) and _p not in sys.path:
        sys.path.append(_p)

from contextlib import ExitStack

import concourse.tile as tile  # noqa: E402
from concourse import bacc, mybir  # noqa: E402
from concourse.bass_utils import run_bass_kernel_spmd  # noqa: E402

P = 128
B, C, H, W = 32, 512, 32, 32
N = H * W                  # 1024 tokens per image
CO = C // P                # 4 channel slabs of 128
FD = 512                   # matmul free-dim chunk (one PSUM bank of fp32)
NCH = N // FD              # 2 free-dim chunks
MO = N // P                # 8 token slabs of 128
GROUPS = 16
EPS = 1e-6
NCORES = 8
IPC = B // NCORES          # images per core
F32 = mybir.dt.float32
F16 = mybir.dt.float16
AF = mybir.ActivationFunctionType
OP = mybir.AluOpType
SCALE = float(C) ** -0.5


def _emit(tc: "tile.TileContext", ctx: ExitStack, aps: dict):
    nc = tc.nc

    const = ctx.enter_context(tc.tile_pool(name="const", bufs=1))
    xs = ctx.enter_context(tc.tile_pool(name="xs", bufs=2))
    hns = ctx.enter_context(tc.tile_pool(name="hns", bufs=2))
    qs = ctx.enter_context(tc.tile_pool(name="qs", bufs=1))
    ks = ctx.enter_context(tc.tile_pool(name="ks", bufs=1))
    vs = ctx.enter_context(tc.tile_pool(name="vs", bufs=1))
    es = ctx.enter_context(tc.tile_pool(name="es", bufs=1))
    ous = ctx.enter_context(tc.tile_pool(name="ous", bufs=1))
    ris = ctx.enter_context(tc.tile_pool(name="ris", bufs=2))
    accp = ctx.enter_context(tc.tile_pool(name="accp", bufs=2))
    ys = ctx.enter_context(tc.tile_pool(name="ys", bufs=3))
    stat = ctx.enter_context(tc.tile_pool(name="stat", bufs=2))
    mmp = ctx.enter_context(tc.tile_pool(name="mmp", bufs=6, space="PSUM"))
    smp = ctx.enter_context(tc.tile_pool(name="smp", bufs=1, space="PSUM"))
    wmp = ctx.enter_context(tc.tile_pool(name="wmp", bufs=1, space="PSUM"))

    # ---- constants: one packed DMA on the GpSimd queue so the Sync queue
    # is free for the critical-path x slabs ----
    ones_sb = const.tile([P, P], F16, tag="ones")
    nc.vector.memset(ones_sb[:], 1.0)
    ones32_sb = const.tile([P, P], mybir.dt.float32r, tag="ones32")
    nc.vector.tensor_copy(ones32_sb[:], ones_sb[:])
    cpack = const.tile([P, 5 * CO + P + C], F32, tag="cpack")
    nc.gpsimd.dma_start(cpack[:], aps["cpack"])
    small = {}
    for i, name in enumerate(("bq", "bk", "bp", "gamma", "beta")):
        small[name] = cpack[:, i * CO : (i + 1) * CO]
    proj_sb = cpack[:, 5 * CO : 5 * CO + P]
    bvb_sb = cpack[:, 5 * CO + P :]

    # Dummy matmuls while groupnorm owns the critical path: PE is idle anyway
    # and sustained activity lifts the HAM clock gate to 8/8 before real work.
    wps = wmp.tile([P, P], F32, tag="warm")

    def warmup(n):
        for i in range(n):
            nc.tensor.matmul(
                wps[:], lhsT=ones_sb[:], rhs=ones_sb[:], start=(i == 0), stop=(i == n - 1)
            )

    w_sb = {}

    def load_weights():
        # Emitted after prep(0) so x(0) slabs go first on the DMA queue;
        # wqT leads since the first projection matmuls consume it.
        for name in ("wqT", "wkT", "wvT", "wpT"):
            t = const.tile([P, CO, C], F16, tag=name)
            nc.sync.dma_start(t[:], aps[name].rearrange("(co ci) o -> ci co o", ci=P))
            w_sb[name] = t

    # Per-image state carried between the pipeline stages below.
    st = [dict() for _ in range(IPC)]

    def prep(img):
        """x DMA + groupnorm -> hn (DVE/ACT work; one tiny PE matmul).

        Emitted one image ahead of its consumer so the DVE/ACT chain overlaps
        the previous image's attention matmuls.  rstd = 1/sqrt(var+eps) runs
        on DVE (quake-style rsqrt + Newton) so the ACT engine only ever needs
        one activation table (exp/copy/identity/square) -> one table load.
        """
        x_ap = aps["x"][img].rearrange("(co ci) n -> ci co n", ci=P)
        x_sb = xs.tile([P, CO, N], F32, tag="x")
        stats = stat.tile([P, 2 * CO], F32, tag="stats")
        for co in range(CO):
            nc.sync.dma_start(x_sb[:, co], x_ap[:, co])
            # sum(x) on DVE, sum(x^2) on ACT (Square + free-dim accumulator)
            # run concurrently; the group projector folds the 1/(32*1024).
            nc.vector.reduce_sum(
                stats[:, co : co + 1], x_sb[:, co], axis=mybir.AxisListType.X
            )
            scr = stat.tile([P, N], F16, tag="sqscr")
            nc.scalar.activation(
                scr[:],
                x_sb[:, co],
                AF.Square,
                accum_out=stats[:, CO + co : CO + co + 1],
            )
        gs_ps = smp.tile([P, 2 * CO], F32, tag="gs")
        nc.tensor.matmul(gs_ps[:], lhsT=proj_sb[:], rhs=stats[:], start=True, stop=True)
        gs = stat.tile([P, 2 * CO], F32, tag="gss")
        nc.scalar.activation(gs[:], gs_ps[:], AF.Copy)
        m2 = stat.tile([P, CO], F32, tag="m2")
        nc.vector.tensor_mul(m2[:], gs[:, 0:CO], gs[:, 0:CO])
        varg = stat.tile([P, CO], F32, tag="varg")
        nc.vector.tensor_sub(varg[:], gs[:, CO : 2 * CO], m2[:])
        # rstd = 1/sqrt(var+eps) entirely on DVE (quake rsqrt + 2 Newton
        # steps, ~5e-6 rel err) so ACT only ever uses the exp table.
        ve = stat.tile([P, CO], F32, tag="ve")
        nc.vector.tensor_scalar(
            out=ve[:], in0=varg[:], scalar1=EPS, scalar2=None, op0=OP.add
        )
        y0i = stat.tile([P, CO], mybir.dt.int32, tag="y0i")
        nc.vector.tensor_scalar(
            out=y0i[:],
            in0=ve[:].bitcast(mybir.dt.int32),
            scalar1=1,
            scalar2=None,
            op0=OP.arith_shift_right,
        )
        nc.vector.tensor_scalar(
            out=y0i[:],
            in0=y0i[:],
            scalar1=-1,
            scalar2=0x5F3759DF,
            op0=OP.mult,
            op1=OP.add,
        )
        rstd = y0i[:].bitcast(F32)
        for _ in range(2):
            yy = stat.tile([P, CO], F32, tag="yy")
            nc.vector.tensor_mul(yy[:], rstd, rstd)
            nc.vector.tensor_mul(yy[:], yy[:], ve[:])
            nc.vector.tensor_scalar(
                out=yy[:], in0=yy[:], scalar1=-0.5, scalar2=1.5, op0=OP.mult, op1=OP.add
            )
            nxt = stat.tile([P, CO], F32, tag="rstd")
            nc.vector.tensor_mul(nxt[:], rstd, yy[:])
            rstd = nxt[:]
        a_sc = stat.tile([P, CO], F32, tag="a_sc")
        nc.vector.tensor_mul(a_sc[:], small["gamma"][:], rstd[:])
        bt = stat.tile([P, CO], F32, tag="bt")
        nc.vector.tensor_mul(bt[:], gs[:, 0:CO], a_sc[:])
        b_sc = stat.tile([P, CO], F32, tag="b_sc")
        nc.vector.tensor_sub(b_sc[:], small["beta"][:], bt[:])

        hn = hns.tile([P, CO, N], F16, tag="hn")
        for co in range(CO):
            nc.vector.tensor_scalar(
                out=hn[:, co],
                in0=x_sb[:, co],
                scalar1=a_sc[:, co : co + 1],
                scalar2=b_sc[:, co : co + 1],
                op0=OP.mult,
                op1=OP.add,
            )
        st[img]["x"] = x_sb
        st[img]["hn"] = hn

    def head(img):
        """q/k projections, vT, scores + exp (the first ~60% of PE work)."""
        hn = st[img]["hn"]
        q_sb = qs.tile([P, CO, N], F16, tag="q")
        k_sb = ks.tile([P, CO, N], F16, tag="k")
        for wname, dst, bname in (("wqT", q_sb, "bq"), ("wkT", k_sb, "bk")):
            wt = w_sb[wname]
            for ot in range(CO):
                for ch in range(NCH):
                    ps = mmp.tile([P, FD], F32, tag="mm")
                    for ci in range(CO):
                        nc.tensor.matmul(
                            ps[:],
                            lhsT=wt[:, ci, ot * P : (ot + 1) * P],
                            rhs=hn[:, ci, ch * FD : (ch + 1) * FD],
                            start=(ci == 0),
                            stop=(ci == CO - 1),
                        )
                    nc.scalar.activation(
                        dst[:, ot, ch * FD : (ch + 1) * FD],
                        ps[:],
                        AF.Identity,
                        bias=small[bname][:, ot : ot + 1],
                    )

        vT = vs.tile([P, MO, C], F16, tag="vT")
        for mt in range(MO):
            ps = mmp.tile([P, FD], F32, tag="mm")
            for ci in range(CO):
                nc.tensor.matmul(
                    ps[:],
                    lhsT=hn[:, ci, mt * P : (mt + 1) * P],
                    rhs=w_sb["wvT"][:, ci, :],
                    start=(ci == 0),
                    stop=(ci == CO - 1),
                )
            nc.vector.tensor_add(vT[:, mt], ps[:], bvb_sb[:])

        eT = es.tile([P, MO, N], F16, tag="eT")
        for mt in range(MO):
            for ch in range(NCH):
                ps = mmp.tile([P, FD], F32, tag="mm")
                for oo in range(CO):
                    nc.tensor.matmul(
                        ps[:],
                        lhsT=k_sb[:, oo, mt * P : (mt + 1) * P],
                        rhs=q_sb[:, oo, ch * FD : (ch + 1) * FD],
                        start=(oo == 0),
                        stop=(oo == CO - 1),
                    )
                nc.scalar.activation(
                    eT[:, mt, ch * FD : (ch + 1) * FD], ps[:], AF.Exp, scale=SCALE
                )
        # Fold the 8 m-slabs of eT on the (otherwise idle) GpSimd engine so the
        # rowsum needs only one ones-matmul per chunk instead of eight.
        acc = accp.tile([P, N], mybir.dt.float32r, tag="acc")
        nc.gpsimd.tensor_add(acc[:], eT[:, 0], eT[:, 1])
        for mt in range(2, MO):
            nc.gpsimd.tensor_add(acc[:], acc[:], eT[:, mt])
        st[img]["vT"] = vT
        st[img]["eT"] = eT
        st[img]["acc"] = acc

    def tail(img):
        """out GEMM, rowsum, proj + residual.  out before rowsum so the PE
        never waits on the last exp evictions; y MMs overlap the reciprocal."""
        x_sb, vT, eT = st[img]["x"], st[img]["vT"], st[img]["eT"]
        y_ap = aps["y"][img].rearrange("(co ci) n -> ci co n", ci=P)

        out_sb = ous.tile([P, CO, N], F16, tag="out")
        for ct in range(CO):
            for ch in range(NCH):
                ps = mmp.tile([P, FD], F32, tag="mm")
                for mt in range(MO):
                    nc.tensor.matmul(
                        ps[:],
                        lhsT=vT[:, mt, ct * P : (ct + 1) * P],
                        rhs=eT[:, mt, ch * FD : (ch + 1) * FD],
                        start=(mt == 0),
                        stop=(mt == MO - 1),
                    )
                nc.scalar.activation(out_sb[:, ct, ch * FD : (ch + 1) * FD], ps[:], AF.Copy)

        acc = st[img]["acc"]
        rinv = ris.tile([P, N], F32, tag="rinv")
        for ch in range(NCH):
            ps = mmp.tile([P, FD], F32, tag="mm")
            nc.tensor.matmul(
                ps[:],
                lhsT=ones32_sb[:],
                rhs=acc[:, ch * FD : (ch + 1) * FD],
                start=True,
                stop=True,
            )
            rscr = ys.tile([P, FD], F32, tag="rscr")
            nc.vector.reciprocal_approx_accurate(
                rinv[:, ch * FD : (ch + 1) * FD], ps[:], rscr[:]
            )

        for ot in range(CO):
            for ch in range(NCH):
                ps = mmp.tile([P, FD], F32, tag="mm")
                for ci in range(CO):
                    nc.tensor.matmul(
                        ps[:],
                        lhsT=w_sb["wpT"][:, ci, ot * P : (ot + 1) * P],
                        rhs=out_sb[:, ci, ch * FD : (ch + 1) * FD],
                        start=(ci == 0),
                        stop=(ci == CO - 1),
                    )
                t1 = ys.tile([P, FD], F32, tag="yt")
                nc.vector.tensor_mul(t1[:], ps[:], rinv[:, ch * FD : (ch + 1) * FD])
                t2 = ys.tile([P, FD], F32, tag="yo")
                nc.vector.scalar_tensor_tensor(
                    out=t2[:],
                    in0=t1[:],
                    scalar=small["bp"][:, ot : ot + 1],
                    in1=x_sb[:, ot, ch * FD : (ch + 1) * FD],
                    op0=OP.add,
                    op1=OP.add,
                )
                nc.sync.dma_start(y_ap[:, ot, ch * FD : (ch + 1) * FD], t2[:])

    warmup(130)
    prep(0)
    warmup(90)
    wsb = stat.tile([P, P], F32, tag="warm_sb")
    nc.scalar.activation(wsb[:], wps[:], AF.Copy)
    nc.gpsimd.dma_start(aps["wsink"], wsb[:])
    load_weights()
    for img in range(IPC):
        head(img)
        if img + 1 < IPC:
            prep(img + 1)
        tail(img)


def _build_program():
    nc = bacc.Bacc("TRN2", target_bir_lowering=False, debug=False)
    aps = {}
    aps["x"] = nc.dram_tensor("x", [IPC, C, N], F32, kind="ExternalInput").ap()
    for name in ("wqT", "wkT", "wvT", "wpT"):
        aps[name] = nc.dram_tensor(name, [C, C], F16, kind="ExternalInput").ap()
    aps["cpack"] = nc.dram_tensor(
        "cpack", [P, 5 * CO + P + C], F32, kind="ExternalInput"
    ).ap()
    aps["y"] = nc.dram_tensor("y", [IPC, C, N], F32, kind="ExternalOutput").ap()
    aps["wsink"] = nc.dram_tensor("wsink", [P, P], F32, kind="ExternalOutput").ap()

    with tile.TileContext(nc) as tc:
        with ExitStack() as ctx:
            _emit(tc, ctx, aps)
    nc.compile()
    return nc


_PROGRAM = None


def _get_program():
    global _PROGRAM
    if _PROGRAM is None:
        _PROGRAM = _build_program()
    return _PROGRAM


def _col_layout(v):
    # (C,) vector -> [128, CO] tile layout with c = co*128 + ci at [ci, co]
    return np.ascontiguousarray(v.reshape(CO, P).T.astype(np.float32))


def _make_in_maps(inputs):
    x = np.asarray(inputs["x"], dtype=np.float32).reshape(B, C, N)
    cpack = np.concatenate(
        [
            _col_layout(np.asarray(inputs["bq"])),
            _col_layout(np.asarray(inputs["bk"])),
            _col_layout(np.asarray(inputs["bp"])),
            _col_layout(np.asarray(inputs["gn_gamma"])),
            _col_layout(np.asarray(inputs["gn_beta"])),
            _make_proj(),
            np.tile(np.asarray(inputs["bv"], dtype=np.float32)[None, :], (P, 1)),
        ],
        axis=1,
    )
    shared = {
        "wqT": np.ascontiguousarray(np.asarray(inputs["wq"]).T.astype(np.float16)),
        "wkT": np.ascontiguousarray(np.asarray(inputs["wk"]).T.astype(np.float16)),
        "wvT": np.ascontiguousarray(np.asarray(inputs["wv"]).T.astype(np.float16)),
        "wpT": np.ascontiguousarray(np.asarray(inputs["wp"]).T.astype(np.float16)),
        "cpack": np.ascontiguousarray(cpack),
    }
    in_maps = []
    for core in range(NCORES):
        m = dict(shared)
        m["x"] = np.ascontiguousarray(x[core * IPC : (core + 1) * IPC])
        in_maps.append(m)
    return in_maps


def _make_proj():
    # [128,128] group-averaging projector applied to raw (sum, sumsq) rows:
    # P[i,j] = (i//32 == j//32) / (32*N)  (channel c = co*128 + ci; each co
    # slab holds 4 groups of 32 channels; stats are sums over N pixels)
    gsz = P // (GROUPS // CO)  # 32
    idx = np.arange(P) // gsz
    return np.ascontiguousarray(
        (idx[:, None] == idx[None, :]).astype(np.float32) / (gsz * N)
    )


def _run(inputs, trace=False):
    nc = _get_program()
    in_maps = _make_in_maps(inputs)
    res = run_bass_kernel_spmd(nc, in_maps, core_ids=list(range(NCORES)), trace=trace)
    y = np.concatenate([r["y"] for r in res.results], axis=0)  # (B, C, N)
    return y.reshape(B, C, H, W).astype(np.float32), res.exec_time_ns


def kernel(**inputs):
    return _run(inputs, trace=False)[0]
